# revision 31
# baseline (speedup 1.0000x reference)
"""Multi-head attention (B=4,S=2048,H=1024,NH=16,D=64) on 8 trn2 cores.

Sharding: core c = (g, b) with g = c // 4 (head-group of 8 heads = 512 dims,
tensor parallel) and b = c % 4 (batch, data parallel). Each core computes a
partial output (its head-group's contribution to the final projection),
transposed: ot = (attn_out_g @ wo_g)^T of shape [H, S]. Host sums the two
group partials per batch and adds bias.

Math notes (host/device split):
  - k-proj bias bk drops out of softmax (adds a per-query constant along the
    key axis), so it is not applied on device.
  - v-proj bias bv commutes through normalized attention (rows of the score
    matrix sum to 1): its contribution is bv @ wo, folded into the output
    bias on the host.

On-device layout: everything is computed transposed (feature dim on
partitions, sequence on the free axis) so the softmax key-axis lands on
partitions. Scores S^T are built per head as K_h^T(stationary) x Q_h^T,
exp() runs on the scalar engine straight out of PSUM, and the ones-column
appended to V in the AV matmul yields the softmax denominators for free.
"""

import sys

if "/opt/trn_rl_repo" not in sys.path:
    sys.path.insert(0, "/opt/trn_rl_repo")

import numpy as np

B, S, H, NH, D = 4, 2048, 1024, 16, 64
G = 2  # head-group split across cores (tensor parallel axis)
GH = H // G  # 512 dims (8 heads) per group
NCORES = 8
SCALE = 1.0 / float(D) ** 0.5  # 1/8

KT = H // 128  # 8 contraction tiles for projections
MT = GH // 128  # 4 m-tiles = head pairs per group
NQC = S // 512  # 4 sequence chunks of 512
SQ = S // 128  # 16 key-sequence tiles
VW = D + 1  # 65: V columns + ones column per head

_CACHE = {}

# build-time tuning knobs (TimelineSim-swept)
CFG = {
    "xs_bufs": 24,
    "w_bufs": 24,
    "pt_bufs": 16,
    "mm_bufs": 2,
    "o_bufs": 2,
    "gpsimd_bcast": True,  # broadcast recip row via gpsimd (else DRAM round-trip)
}


def _build():
    import concourse.tile as tile
    from concourse import bacc, mybir

    F32 = mybir.dt.float32
    F32R = mybir.dt.float16  # all-f16 variant: f16 matmuls everywhere
    BF16 = mybir.dt.float16  # f16: same PE speed as bf16, 3 more mantissa bits
    AF = mybir.ActivationFunctionType
    OP = mybir.AluOpType

    nc = bacc.Bacc("TRN2", target_bir_lowering=False, debug=False)

    xq = nc.dram_tensor("xq", [H, S], F32R, kind="ExternalInput")
    xk = nc.dram_tensor("xk", [H, S], F32R, kind="ExternalInput")
    xv = nc.dram_tensor("xv", [H, S], F32R, kind="ExternalInput")
    wqd = nc.dram_tensor("wq", [H, GH], F32R, kind="ExternalInput")
    wkd = nc.dram_tensor("wk", [H, GH], F32R, kind="ExternalInput")
    wvd = nc.dram_tensor("wv", [H, GH], F32R, kind="ExternalInput")
    wod = nc.dram_tensor("wo", [GH, H], F32, kind="ExternalInput")
    bqd = nc.dram_tensor("bq", [GH], F32, kind="ExternalInput")
    otd = nc.dram_tensor("ot", [H, S], F32R, kind="ExternalOutput")

    with tile.TileContext(nc) as tc:
        with (
            tc.tile_pool(name="res", bufs=1) as res,
            tc.tile_pool(name="rot", bufs=2) as rot,
            tc.tile_pool(name="psmm", bufs=CFG["mm_bufs"], space="PSUM") as psmm,
            tc.tile_pool(name="pso", bufs=CFG["o_bufs"], space="PSUM") as pso,
            tc.tile_pool(name="dsc", bufs=4, space="DRAM") as dsc,
        ):
            # ---- residents ----
            qhT = [
                res.tile([128, S], F32R, tag=f"qhT{m}", name=f"qhT{m}")
                for m in range(MT)
            ]
            khT = [
                res.tile([128, S], F32R, tag=f"khT{m}", name=f"khT{m}")
                for m in range(MT)
            ]
            oT = [
                res.tile([128, S], BF16, tag=f"oT{t}", name=f"oT{t}")
                for t in range(MT)
            ]
            vaug = res.tile([128, SQ * 8 * VW], BF16, tag="vaug", name="vaug")
            wo_bf = [
                res.tile([128, H], BF16, tag=f"wob{t}", name=f"wob{t}")
                for t in range(MT)
            ]
            bq_sb = res.tile([128, MT], F32, tag="bqsb", name="bq_sb")

            # ---- constants / weights staging ----
            for m in range(MT):
                nc.sync.dma_start(
                    out=bq_sb[:, m : m + 1],
                    in_=bqd.ap()[m * 128 : (m + 1) * 128].rearrange(
                        "(p o) -> p o", o=1
                    ),
                )
            for t in range(MT):
                wos = rot.tile([128, H], F32, tag="wos", bufs=2, name=f"wos{t}")
                nc.sync.dma_start(out=wos, in_=wod.ap()[t * 128 : (t + 1) * 128, :])
                nc.vector.tensor_copy(wo_bf[t], wos)
            # ones columns of vaug (V slots are overwritten by the V proj)
            nc.vector.memset(vaug, 1.0)

            def load_w(wd):
                ws = []
                for kt in range(KT):
                    wt = rot.tile([128, GH], F32R, tag="w", bufs=CFG["w_bufs"], name=f"w{kt}")
                    nc.sync.dma_start(out=wt, in_=wd.ap()[kt * 128 : (kt + 1) * 128, :])
                    ws.append(wt)
                return ws

            def load_strips(xd, qc):
                xs = []
                for kt in range(KT):
                    st = rot.tile([128, 512], F32R, tag="xs", bufs=CFG["xs_bufs"], name=f"xs{kt}")
                    nc.sync.dma_start(
                        out=st,
                        in_=xd.ap()[
                            kt * 128 : (kt + 1) * 128, qc * 512 : (qc + 1) * 512
                        ],
                    )
                    xs.append(st)
                return xs

            # ---- chunked projections (issued piecemeal between/inside
            # attention blocks so they execute in the tensor engine's slack
            # while the scalar engine grinds exps) ----
            wv_sb = wk_sb = wq_sb = None
            strip_cache = {}

            def strips_for(key, xd, qc):
                if key not in strip_cache:
                    strip_cache[key] = load_strips(xd, qc)
                return strip_cache[key]

            def v_chunk(qc, sql):
                # one [128 seq, 512 dims] V-proj chunk -> vaug columns
                xs = strips_for(("v", qc), xv, qc)
                sq = qc * 4 + sql
                ps = psmm.tile([128, 512], F32, tag="mm", name=f"psv{sq}")
                for kt in range(KT):
                    nc.tensor.matmul(
                        ps,
                        lhsT=xs[kt][:, sql * 128 : (sql + 1) * 128],
                        rhs=wv_sb[kt],
                        start=(kt == 0),
                        stop=(kt == KT - 1),
                    )
                base = sq * 8 * VW
                for h in range(8):
                    nc.vector.tensor_copy(
                        vaug[:, base + h * VW : base + h * VW + D],
                        ps[:, h * D : (h + 1) * D],
                    )

            def k_chunk(qc, m, pool=None):
                xs = strips_for(("k", qc), xk, qc)
                p = pool or psmm
                ps = p.tile([128, 512], F32, tag=("mm" if p is psmm else "o"), name=f"psk{m}")
                for kt in range(KT):
                    nc.tensor.matmul(
                        ps,
                        lhsT=wk_sb[kt][:, m * 128 : (m + 1) * 128],
                        rhs=xs[kt],
                        start=(kt == 0),
                        stop=(kt == KT - 1),
                    )
                nc.vector.tensor_copy(khT[m][:, qc * 512 : (qc + 1) * 512], ps)

            def q_chunk(qc, m, pool=None):
                xs = strips_for(("q", qc), xq, qc)
                p = pool or psmm
                ps = p.tile([128, 512], F32, tag=("mm" if p is psmm else "o"), name=f"psq{m}")
                for kt in range(KT):
                    nc.tensor.matmul(
                        ps,
                        lhsT=wq_sb[kt][:, m * 128 : (m + 1) * 128],
                        rhs=xs[kt],
                        start=(kt == 0),
                        stop=(kt == KT - 1),
                    )
                nc.vector.tensor_scalar(
                    qhT[m][:, qc * 512 : (qc + 1) * 512],
                    ps,
                    bq_sb[:, m : m + 1],
                    None,
                    OP.add,
                )

            def release_strips(key):
                strip_cache.pop(key, None)

            def lowprio(thunk, bump=1 << 20):
                # issue with priorities far in the future: the scheduler
                # dispatches these only into true slack — attention always
                # wins contested PSUM slots and engine time
                def run():
                    p = tc.cur_priority
                    tc.cur_priority = p + bump
                    thunk()
                    tc.cur_priority = p
                return run

            # ---- attention for head pair t over a 1024-wide query chunk ----
            # hooks: {kt: [thunk, ...]} — projection chunks issued at the top
            # of the kt iteration so they fill tensor slack without breaking
            # sequential dependency semantics
            def attention(t, qcp, hooks=None, pend_depth=3):
                q0 = qcp * 1024
                ps_o = [
                    pso.tile([VW, 1024], F32, tag="o", name=f"pso{hh}")
                    for hh in range(2)
                ]
                pending = []  # (kt, [pt_qch0, pt_qch1]) awaiting AV
                def emit_av(kt, pts):
                    for hh in range(2):
                        h_abs = 2 * t + hh
                        vbase = kt * 8 * VW + h_abs * VW
                        for qch in range(2):
                            nc.tensor.matmul(
                                ps_o[hh][:, qch * 512 : (qch + 1) * 512],
                                lhsT=vaug[:, vbase : vbase + VW],
                                rhs=pts[qch][:, hh * 512 : (hh + 1) * 512],
                                start=(kt == 0),
                                stop=(kt == SQ - 1),
                            )
                for kt in range(SQ):
                    if hooks and kt in hooks:
                        for thunk in hooks[kt]:
                            thunk()
                    # per query-half, BOTH heads' scores go into one shared
                    # PSUM tile (h0 -> bank 0 cols, h64 -> bank 1 cols): the
                    # pair becomes ready at the same instant, dispatches
                    # adjacently, and runs concurrently in disjoint PE
                    # row-groups (auto tile_position from base partition)
                    pts = []
                    for qch in range(2):
                        ps_s = psmm.tile([128, 1024], F32, tag="mm", name="pss")
                        for hh in range(2):
                            hp = 64 * hh
                            nc.tensor.matmul(
                                ps_s[:, hh * 512 : (hh + 1) * 512],
                                lhsT=khT[t][hp : hp + 64, kt * 128 : (kt + 1) * 128],
                                rhs=qhT[t][
                                    hp : hp + 64, q0 + qch * 512 : q0 + (qch + 1) * 512
                                ],
                                start=True,
                                stop=True,
                            )
                        pt_t = rot.tile([128, 1024], BF16, tag="pt", bufs=CFG["pt_bufs"], name="pt")
                        nc.scalar.activation(pt_t, ps_s, AF.Exp, scale=SCALE)
                        pts.append(pt_t)
                    pending.append((kt, pts))
                    if len(pending) > pend_depth:
                        emit_av(*pending.pop(0))
                for p in pending:
                    emit_av(*p)
                # normalize by the ones-column sums; heads stack on partitions
                for hh in range(2):
                    # drain the PSUM accumulator to SBUF immediately so the
                    # next head pair's AV can claim the banks while
                    # normalization is still in flight; the denom row DMAs to
                    # partition 0 (recip_approx_fast and partition_broadcast
                    # silently read partition 0 on HW whatever the AP says)
                    ou = rot.tile([VW, 1024], F32, tag="ou", bufs=2, name="ou")
                    nc.vector.tensor_copy(ou, ps_o[hh])
                    # the rest of the chain is latency-tolerant: issue at low
                    # priority so projection-chunk copies in the DVE FIFO
                    # (which gate PSUM slot releases) dispatch first
                    pri = tc.cur_priority
                    tc.cur_priority = pri + (1 << 20)
                    # rcp half lives at column 0 (partition_broadcast reads
                    # the tile base address)
                    dr = rot.tile([1, 2048], F32, tag="dr", bufs=2, name="dr")
                    den = dr[:, 1024:2048]
                    rcp = dr[:, 0:1024]
                    nc.sync.dma_start(out=den, in_=ou[D : D + 1, :])
                    nc.vector.reciprocal_approx_fast(rcp, den)
                    bc = rot.tile([64, 1024], F32, tag="bc", bufs=2, name="bc")
                    if CFG["gpsimd_bcast"]:
                        nc.gpsimd.partition_broadcast(bc, rcp)
                    else:
                        # DRAM round-trip (DRAM source DMA supports 0-stride
                        # partition reads)
                        sc = dsc.tile([1, 1024], F32, tag="sc", name="sc")
                        nc.sync.dma_start(out=sc, in_=rcp)
                        nc.sync.dma_start(
                            out=bc, in_=sc[0, :].partition_broadcast(64)
                        )
                    if hh == 0:
                        nc.vector.tensor_tensor(
                            oT[t][0:64, q0 : q0 + 1024],
                            ou[0:D, :],
                            bc,
                            OP.mult,
                        )
                    else:
                        # normalized h1 lands on partitions 0-63; DMA shifts it
                        # onto partitions 64-127 of the head-pair tile
                        otn = rot.tile([64, 1024], BF16, tag="otn", bufs=2, name="otn")
                        nc.vector.tensor_tensor(otn, ou[0:D, :], bc, OP.mult)
                        nc.sync.dma_start(
                            out=oT[t][64:128, q0 : q0 + 1024], in_=otn
                        )
                    tc.cur_priority = pri

            # ---- output projection for one 512-wide sequence chunk ----
            # f16 output halves the write traffic; at the tail the scalar
            # engine is idle so its HWDGE queue doubles the drain bandwidth
            def out_proj(qcc, spread=False, pool=None):
                for m in range(H // 128):
                    p = pool or (pso if m % 2 else psmm)
                    ps = p.tile([128, 512], F32, tag=("mm" if p is psmm else "o"), name=f"pso{m}")
                    for t in range(MT):
                        nc.tensor.matmul(
                            ps,
                            lhsT=wo_bf[t][:, m * 128 : (m + 1) * 128],
                            rhs=oT[t][:, qcc * 512 : (qcc + 1) * 512],
                            start=(t == 0),
                            stop=(t == MT - 1),
                        )
                    osb = rot.tile([128, 512], F32R, tag="osb", bufs=4, name="osb")
                    nc.vector.tensor_copy(osb, ps)
                    eng = nc.scalar if (spread and m % 2) else nc.sync
                    eng.dma_start(
                        out=otd.ap()[m * 128 : (m + 1) * 128, qcc * 512 : (qcc + 1) * 512],
                        in_=osb,
                    )

            # ---- main sequence: minimal prefix (K cols 0:1024 + Q m0),
            # then attention blocks with projection chunks interleaved via
            # hooks into the scalar-bound kt loops ----
            wk_sb = load_w(wkd)
            for qc in range(NQC):
                for m in range(MT):
                    k_chunk(qc, m, pool=(pso if m % 2 else psmm))
                release_strips(("k", qc))
            wq_sb = load_w(wqd)
            for qc in range(2):
                for m in range(MT):
                    q_chunk(qc, m, pool=(pso if m % 2 else psmm))
                release_strips(("q", qc))
            wv_sb = load_w(wvd)

            # block (0,0) hooks: V chunks paced just ahead of their AV
            # consumers, filling tensor slack in the scalar-bound kt loop
            hooks00 = {
                0: [lowprio(lambda: v_chunk(0, 0))],
                1: [lowprio(lambda: v_chunk(0, 1))],
                2: [lowprio(lambda: v_chunk(0, 2))],
                3: [lowprio(lambda: v_chunk(0, 3))],
                4: [lowprio(lambda: v_chunk(1, 0))],
                5: [lowprio(lambda: v_chunk(1, 1))],
                6: [lowprio(lambda: v_chunk(1, 2))],
                7: [lowprio(lambda: v_chunk(1, 3))],
                8: [lowprio(lambda: v_chunk(2, 0))],
                9: [lowprio(lambda: v_chunk(2, 1))],
                10: [lowprio(lambda: v_chunk(2, 2))],
                11: [lowprio(lambda: v_chunk(2, 3))],
                12: [lowprio(lambda: v_chunk(3, 0))],
                13: [lowprio(lambda: v_chunk(3, 1)), lowprio(lambda: v_chunk(3, 2))],
                14: [lowprio(lambda: v_chunk(3, 3))],
            }
            attention(0, 0, hooks00)
            attention(1, 0)
            attention(2, 0)
            attention(3, 0)
            for m in range(MT):
                q_chunk(2, m)
                q_chunk(3, m, pool=pso)
            attention(0, 1)
            out_proj(0)
            attention(1, 1)
            out_proj(1)
            attention(2, 1)
            attention(3, 1, pend_depth=1)
            # tail: all PSUM banks are free — run the two final chunks from
            # separate pools so four m-tiles pipeline concurrently
            out_proj(2, spread=True)
            out_proj(3, spread=True)

    nc.compile()
    return nc


def _get_nc():
    if "nc" not in _CACHE:
        _CACHE["nc"] = _build()
    return _CACHE["nc"]


def make_in_maps(q, k, v, wq, wk, wv, wo, bq):
    q = np.asarray(q, np.float32)
    k = np.asarray(k, np.float32)
    v = np.asarray(v, np.float32)
    in_maps = []
    for c in range(NCORES):
        g, b = divmod(c, B)
        sl = slice(g * GH, (g + 1) * GH)
        in_maps.append(
            {
                "xq": np.ascontiguousarray(q[b].T).astype(np.float16),
                "xk": np.ascontiguousarray(k[b].T).astype(np.float16),
                "xv": np.ascontiguousarray(v[b].T).astype(np.float16),
                "wq": np.ascontiguousarray(np.asarray(wq, np.float32)[:, sl]).astype(np.float16),
                "wk": np.ascontiguousarray(np.asarray(wk, np.float32)[:, sl]).astype(np.float16),
                "wv": np.ascontiguousarray(np.asarray(wv, np.float32)[:, sl]).astype(np.float16),
                "wo": np.ascontiguousarray(np.asarray(wo, np.float32)[sl, :]),
                "bq": np.ascontiguousarray(np.asarray(bq, np.float32)[sl]),
            }
        )
    return in_maps


def assemble(per_core_ot, bv, wo, bo):
    bo_eff = (
        np.asarray(bo, np.float32)
        + np.asarray(bv, np.float32) @ np.asarray(wo, np.float32)
    )
    out = np.empty((B, S, H), np.float32)
    for b in range(B):
        out[b] = (
            per_core_ot[b].T.astype(np.float32)
            + per_core_ot[B + b].T.astype(np.float32)
            + bo_eff
        )
    return out


def kernel(q, k, v, wq, bq, wk, bk, wv, bv, wo, bo, _trace=False):
    from concourse.bass_utils import run_bass_kernel_spmd

    nc = _get_nc()
    in_maps = make_in_maps(q, k, v, wq, wk, wv, wo, bq)
    res = run_bass_kernel_spmd(
        nc, in_maps, core_ids=list(range(NCORES)), trace=_trace
    )
    _CACHE["last_results"] = res
    outs = [res.results[c]["ot"] for c in range(NCORES)]
    return assemble(outs, bv, wo, bo)



# revision 34
# speedup vs baseline: 1.0192x; 1.0192x over previous
"""Multi-head attention (B=4,S=2048,H=1024,NH=16,D=64) on 8 trn2 cores.

Sharding: core c = (g, b) with g = c // 4 (head-group of 8 heads = 512 dims,
tensor parallel) and b = c % 4 (batch, data parallel). Each core computes a
partial output (its head-group's contribution to the final projection),
transposed: ot = (attn_out_g @ wo_g)^T of shape [H, S]. Host sums the two
group partials per batch and adds bias.

Math notes (host/device split):
  - k-proj bias bk drops out of softmax (adds a per-query constant along the
    key axis), so it is not applied on device.
  - v-proj bias bv commutes through normalized attention (rows of the score
    matrix sum to 1): its contribution is bv @ wo, folded into the output
    bias on the host.

On-device layout: everything is computed transposed (feature dim on
partitions, sequence on the free axis) so the softmax key-axis lands on
partitions. Scores S^T are built per head as K_h^T(stationary) x Q_h^T,
exp() runs on the scalar engine straight out of PSUM, and the ones-column
appended to V in the AV matmul yields the softmax denominators for free.
"""

import sys

if "/opt/trn_rl_repo" not in sys.path:
    sys.path.insert(0, "/opt/trn_rl_repo")

import numpy as np

B, S, H, NH, D = 4, 2048, 1024, 16, 64
G = 2  # head-group split across cores (tensor parallel axis)
GH = H // G  # 512 dims (8 heads) per group
NCORES = 8
SCALE = 1.0 / float(D) ** 0.5  # 1/8

KT = H // 128  # 8 contraction tiles for projections
MT = GH // 128  # 4 m-tiles = head pairs per group
NQC = S // 512  # 4 sequence chunks of 512
SQ = S // 128  # 16 key-sequence tiles
VW = D + 1  # 65: V columns + ones column per head

_CACHE = {}

# build-time tuning knobs (TimelineSim-swept)
CFG = {
    "xs_bufs": 24,
    "w_bufs": 24,
    "pt_bufs": 16,
    "mm_bufs": 2,
    "o_bufs": 2,
    "gpsimd_bcast": True,  # broadcast recip row via gpsimd (else DRAM round-trip)
}


def _build():
    import concourse.tile as tile
    from concourse import bacc, mybir

    F32 = mybir.dt.float32
    F32R = mybir.dt.float16  # all-f16 variant: f16 matmuls everywhere
    BF16 = mybir.dt.float16  # f16: same PE speed as bf16, 3 more mantissa bits
    AF = mybir.ActivationFunctionType
    OP = mybir.AluOpType

    nc = bacc.Bacc("TRN2", target_bir_lowering=False, debug=False)

    xq = nc.dram_tensor("xq", [H, S], F32R, kind="ExternalInput")
    xk = nc.dram_tensor("xk", [H, S], F32R, kind="ExternalInput")
    xv = nc.dram_tensor("xv", [H, S], F32R, kind="ExternalInput")
    wqd = nc.dram_tensor("wq", [H, GH], F32R, kind="ExternalInput")
    wkd = nc.dram_tensor("wk", [H, GH], F32R, kind="ExternalInput")
    wvd = nc.dram_tensor("wv", [H, GH], F32R, kind="ExternalInput")
    wod = nc.dram_tensor("wo", [GH, H], F32, kind="ExternalInput")
    bqd = nc.dram_tensor("bq", [GH], F32, kind="ExternalInput")
    otd = nc.dram_tensor("ot", [H, S], F32R, kind="ExternalOutput")

    with tile.TileContext(nc) as tc:
        with (
            tc.tile_pool(name="res", bufs=1) as res,
            tc.tile_pool(name="rot", bufs=2) as rot,
            tc.tile_pool(name="psmm", bufs=CFG["mm_bufs"], space="PSUM") as psmm,
            tc.tile_pool(name="pso", bufs=CFG["o_bufs"], space="PSUM") as pso,
            tc.tile_pool(name="dsc", bufs=4, space="DRAM") as dsc,
        ):
            # ---- residents ----
            qhT = [
                res.tile([128, S], F32R, tag=f"qhT{m}", name=f"qhT{m}")
                for m in range(MT)
            ]
            khT = [
                res.tile([128, S], F32R, tag=f"khT{m}", name=f"khT{m}")
                for m in range(MT)
            ]
            oT = [
                res.tile([128, S], BF16, tag=f"oT{t}", name=f"oT{t}")
                for t in range(MT)
            ]
            vaug = res.tile([128, SQ * 8 * VW], BF16, tag="vaug", name="vaug")
            wo_bf = [
                res.tile([128, H], BF16, tag=f"wob{t}", name=f"wob{t}")
                for t in range(MT)
            ]
            bq_sb = res.tile([128, MT], F32, tag="bqsb", name="bq_sb")

            # ---- constants / weights staging ----
            for m in range(MT):
                nc.sync.dma_start(
                    out=bq_sb[:, m : m + 1],
                    in_=bqd.ap()[m * 128 : (m + 1) * 128].rearrange(
                        "(p o) -> p o", o=1
                    ),
                )
            for t in range(MT):
                wos = rot.tile([128, H], F32, tag="wos", bufs=2, name=f"wos{t}")
                nc.sync.dma_start(out=wos, in_=wod.ap()[t * 128 : (t + 1) * 128, :])
                nc.vector.tensor_copy(wo_bf[t], wos)
            # ones columns of vaug (V slots are overwritten by the V proj)
            nc.vector.memset(vaug, 1.0)

            def load_w(wd):
                ws = []
                for kt in range(KT):
                    wt = rot.tile([128, GH], F32R, tag="w", bufs=CFG["w_bufs"], name=f"w{kt}")
                    nc.sync.dma_start(out=wt, in_=wd.ap()[kt * 128 : (kt + 1) * 128, :])
                    ws.append(wt)
                return ws

            def load_strips(xd, qc):
                xs = []
                for kt in range(KT):
                    st = rot.tile([128, 512], F32R, tag="xs", bufs=CFG["xs_bufs"], name=f"xs{kt}")
                    nc.sync.dma_start(
                        out=st,
                        in_=xd.ap()[
                            kt * 128 : (kt + 1) * 128, qc * 512 : (qc + 1) * 512
                        ],
                    )
                    xs.append(st)
                return xs

            # ---- chunked projections (issued piecemeal between/inside
            # attention blocks so they execute in the tensor engine's slack
            # while the scalar engine grinds exps) ----
            wv_sb = wk_sb = wq_sb = None
            strip_cache = {}

            def strips_for(key, xd, qc):
                if key not in strip_cache:
                    strip_cache[key] = load_strips(xd, qc)
                return strip_cache[key]

            def v_chunk(qc, sql):
                # one [128 seq, 512 dims] V-proj chunk -> vaug columns
                xs = strips_for(("v", qc), xv, qc)
                sq = qc * 4 + sql
                ps = psmm.tile([128, 512], F32, tag="mm", name=f"psv{sq}")
                for kt in range(KT):
                    nc.tensor.matmul(
                        ps,
                        lhsT=xs[kt][:, sql * 128 : (sql + 1) * 128],
                        rhs=wv_sb[kt],
                        start=(kt == 0),
                        stop=(kt == KT - 1),
                    )
                # single strided copy into the 8 stride-VW head slots: one
                # DVE op instead of 8 releases the PSUM slot ~1us sooner
                base = sq * 8 * VW
                nc.vector.tensor_copy(
                    vaug[:, base : base + 8 * VW].rearrange(
                        "p (h w) -> p h w", w=VW
                    )[:, :, 0:D],
                    ps.rearrange("p (h w) -> p h w", h=8),
                )

            def k_chunk(qc, m, pool=None):
                xs = strips_for(("k", qc), xk, qc)
                p = pool or psmm
                ps = p.tile([128, 512], F32, tag=("mm" if p is psmm else "o"), name=f"psk{m}")
                for kt in range(KT):
                    nc.tensor.matmul(
                        ps,
                        lhsT=wk_sb[kt][:, m * 128 : (m + 1) * 128],
                        rhs=xs[kt],
                        start=(kt == 0),
                        stop=(kt == KT - 1),
                    )
                nc.vector.tensor_copy(khT[m][:, qc * 512 : (qc + 1) * 512], ps)

            def q_chunk(qc, m, pool=None):
                xs = strips_for(("q", qc), xq, qc)
                p = pool or psmm
                ps = p.tile([128, 512], F32, tag=("mm" if p is psmm else "o"), name=f"psq{m}")
                for kt in range(KT):
                    nc.tensor.matmul(
                        ps,
                        lhsT=wq_sb[kt][:, m * 128 : (m + 1) * 128],
                        rhs=xs[kt],
                        start=(kt == 0),
                        stop=(kt == KT - 1),
                    )
                nc.vector.tensor_scalar(
                    qhT[m][:, qc * 512 : (qc + 1) * 512],
                    ps,
                    bq_sb[:, m : m + 1],
                    None,
                    OP.add,
                )

            def release_strips(key):
                strip_cache.pop(key, None)

            def lowprio(thunk, bump=1 << 20):
                # issue with priorities far in the future: the scheduler
                # dispatches these only into true slack — attention always
                # wins contested PSUM slots and engine time
                def run():
                    p = tc.cur_priority
                    tc.cur_priority = p + bump
                    thunk()
                    tc.cur_priority = p
                return run

            # ---- attention for head pair t over a 1024-wide query chunk ----
            # hooks: {kt: [thunk, ...]} — projection chunks issued at the top
            # of the kt iteration so they fill tensor slack without breaking
            # sequential dependency semantics
            def attention(t, qcp, hooks=None, pend_depth=3):
                q0 = qcp * 1024
                ps_o = [
                    pso.tile([VW, 1024], F32, tag="o", name=f"pso{hh}")
                    for hh in range(2)
                ]
                pending = []  # (kt, [pt_qch0, pt_qch1]) awaiting AV
                def emit_av(kt, pts):
                    for hh in range(2):
                        h_abs = 2 * t + hh
                        vbase = kt * 8 * VW + h_abs * VW
                        for qch in range(2):
                            nc.tensor.matmul(
                                ps_o[hh][:, qch * 512 : (qch + 1) * 512],
                                lhsT=vaug[:, vbase : vbase + VW],
                                rhs=pts[qch][:, hh * 512 : (hh + 1) * 512],
                                start=(kt == 0),
                                stop=(kt == SQ - 1),
                            )
                for kt in range(SQ):
                    if hooks and kt in hooks:
                        for thunk in hooks[kt]:
                            thunk()
                    # per query-half, BOTH heads' scores go into one shared
                    # PSUM tile (h0 -> bank 0 cols, h64 -> bank 1 cols): the
                    # pair becomes ready at the same instant, dispatches
                    # adjacently, and runs concurrently in disjoint PE
                    # row-groups (auto tile_position from base partition)
                    pts = []
                    for qch in range(2):
                        ps_s = psmm.tile([128, 1024], F32, tag="mm", name="pss")
                        for hh in range(2):
                            hp = 64 * hh
                            nc.tensor.matmul(
                                ps_s[:, hh * 512 : (hh + 1) * 512],
                                lhsT=khT[t][hp : hp + 64, kt * 128 : (kt + 1) * 128],
                                rhs=qhT[t][
                                    hp : hp + 64, q0 + qch * 512 : q0 + (qch + 1) * 512
                                ],
                                start=True,
                                stop=True,
                            )
                        pt_t = rot.tile([128, 1024], BF16, tag="pt", bufs=CFG["pt_bufs"], name="pt")
                        nc.scalar.activation(pt_t, ps_s, AF.Exp, scale=SCALE)
                        pts.append(pt_t)
                    pending.append((kt, pts))
                    if len(pending) > pend_depth:
                        emit_av(*pending.pop(0))
                for p in pending:
                    emit_av(*p)
                # normalize by the ones-column sums; heads stack on partitions
                for hh in range(2):
                    # drain the PSUM accumulator to SBUF immediately so the
                    # next head pair's AV can claim the banks while
                    # normalization is still in flight; the denom row DMAs to
                    # partition 0 (recip_approx_fast and partition_broadcast
                    # silently read partition 0 on HW whatever the AP says)
                    ou = rot.tile([VW, 1024], F32, tag="ou", bufs=2, name="ou")
                    nc.vector.tensor_copy(ou, ps_o[hh])
                    # the rest of the chain is latency-tolerant: issue at low
                    # priority so projection-chunk copies in the DVE FIFO
                    # (which gate PSUM slot releases) dispatch first
                    pri = tc.cur_priority
                    tc.cur_priority = pri + (1 << 20)
                    # rcp half lives at column 0 (partition_broadcast reads
                    # the tile base address)
                    dr = rot.tile([1, 2048], F32, tag="dr", bufs=2, name="dr")
                    den = dr[:, 1024:2048]
                    rcp = dr[:, 0:1024]
                    nc.sync.dma_start(out=den, in_=ou[D : D + 1, :])
                    nc.vector.reciprocal_approx_fast(rcp, den)
                    bc = rot.tile([64, 1024], F32, tag="bc", bufs=2, name="bc")
                    if CFG["gpsimd_bcast"]:
                        nc.gpsimd.partition_broadcast(bc, rcp)
                    else:
                        # DRAM round-trip (DRAM source DMA supports 0-stride
                        # partition reads)
                        sc = dsc.tile([1, 1024], F32, tag="sc", name="sc")
                        nc.sync.dma_start(out=sc, in_=rcp)
                        nc.sync.dma_start(
                            out=bc, in_=sc[0, :].partition_broadcast(64)
                        )
                    if hh == 0:
                        nc.vector.tensor_tensor(
                            oT[t][0:64, q0 : q0 + 1024],
                            ou[0:D, :],
                            bc,
                            OP.mult,
                        )
                    else:
                        # normalized h1 lands on partitions 0-63; DMA shifts it
                        # onto partitions 64-127 of the head-pair tile
                        otn = rot.tile([64, 1024], BF16, tag="otn", bufs=2, name="otn")
                        nc.vector.tensor_tensor(otn, ou[0:D, :], bc, OP.mult)
                        nc.sync.dma_start(
                            out=oT[t][64:128, q0 : q0 + 1024], in_=otn
                        )
                    tc.cur_priority = pri

            # ---- output projection for one 512-wide sequence chunk ----
            # f16 output halves the write traffic; at the tail the scalar
            # engine is idle so its HWDGE queue doubles the drain bandwidth
            def out_proj(qcc, spread=False, pool=None):
                for m in range(H // 128):
                    p = pool or (pso if m % 2 else psmm)
                    ps = p.tile([128, 512], F32, tag=("mm" if p is psmm else "o"), name=f"pso{m}")
                    for t in range(MT):
                        nc.tensor.matmul(
                            ps,
                            lhsT=wo_bf[t][:, m * 128 : (m + 1) * 128],
                            rhs=oT[t][:, qcc * 512 : (qcc + 1) * 512],
                            start=(t == 0),
                            stop=(t == MT - 1),
                        )
                    osb = rot.tile([128, 512], F32R, tag="osb", bufs=4, name="osb")
                    nc.vector.tensor_copy(osb, ps)
                    eng = nc.scalar if (spread and m % 2) else nc.sync
                    eng.dma_start(
                        out=otd.ap()[m * 128 : (m + 1) * 128, qcc * 512 : (qcc + 1) * 512],
                        in_=osb,
                    )

            # ---- main sequence: minimal prefix (K cols 0:1024 + Q m0),
            # then attention blocks with projection chunks interleaved via
            # hooks into the scalar-bound kt loops ----
            wk_sb = load_w(wkd)
            for qc in range(NQC):
                for m in range(MT):
                    k_chunk(qc, m, pool=(pso if m % 2 else psmm))
                release_strips(("k", qc))
            wq_sb = load_w(wqd)
            for qc in range(2):
                for m in range(MT):
                    q_chunk(qc, m, pool=(pso if m % 2 else psmm))
                release_strips(("q", qc))
            wv_sb = load_w(wvd)

            # block (0,0) hooks: V chunks paced just ahead of their AV
            # consumers, filling tensor slack in the scalar-bound kt loop
            hooks00 = {
                0: [lowprio(lambda: v_chunk(0, 0))],
                1: [lowprio(lambda: v_chunk(0, 1))],
                2: [lowprio(lambda: v_chunk(0, 2))],
                3: [lowprio(lambda: v_chunk(0, 3))],
                4: [lowprio(lambda: v_chunk(1, 0))],
                5: [lowprio(lambda: v_chunk(1, 1))],
                6: [lowprio(lambda: v_chunk(1, 2))],
                7: [lowprio(lambda: v_chunk(1, 3))],
                8: [lowprio(lambda: v_chunk(2, 0))],
                9: [lowprio(lambda: v_chunk(2, 1))],
                10: [lowprio(lambda: v_chunk(2, 2))],
                11: [lowprio(lambda: v_chunk(2, 3))],
                12: [lowprio(lambda: v_chunk(3, 0))],
                13: [lowprio(lambda: v_chunk(3, 1)), lowprio(lambda: v_chunk(3, 2))],
                14: [lowprio(lambda: v_chunk(3, 3))],
            }
            attention(0, 0, hooks00)
            attention(1, 0)
            attention(2, 0)
            attention(3, 0)
            for m in range(MT):
                q_chunk(2, m)
                q_chunk(3, m, pool=pso)
            attention(0, 1)
            out_proj(0)
            attention(1, 1)
            out_proj(1)
            attention(2, 1)
            attention(3, 1, pend_depth=1)
            # tail: all PSUM banks are free — run the two final chunks from
            # separate pools so four m-tiles pipeline concurrently
            out_proj(2, spread=True)
            out_proj(3, spread=True)

    nc.compile()
    return nc


def _get_nc():
    if "nc" not in _CACHE:
        _CACHE["nc"] = _build()
    return _CACHE["nc"]


def make_in_maps(q, k, v, wq, wk, wv, wo, bq):
    q = np.asarray(q, np.float32)
    k = np.asarray(k, np.float32)
    v = np.asarray(v, np.float32)
    in_maps = []
    for c in range(NCORES):
        g, b = divmod(c, B)
        sl = slice(g * GH, (g + 1) * GH)
        in_maps.append(
            {
                "xq": np.ascontiguousarray(q[b].T).astype(np.float16),
                "xk": np.ascontiguousarray(k[b].T).astype(np.float16),
                "xv": np.ascontiguousarray(v[b].T).astype(np.float16),
                "wq": np.ascontiguousarray(np.asarray(wq, np.float32)[:, sl]).astype(np.float16),
                "wk": np.ascontiguousarray(np.asarray(wk, np.float32)[:, sl]).astype(np.float16),
                "wv": np.ascontiguousarray(np.asarray(wv, np.float32)[:, sl]).astype(np.float16),
                "wo": np.ascontiguousarray(np.asarray(wo, np.float32)[sl, :]),
                "bq": np.ascontiguousarray(np.asarray(bq, np.float32)[sl]),
            }
        )
    return in_maps


def assemble(per_core_ot, bv, wo, bo):
    bo_eff = (
        np.asarray(bo, np.float32)
        + np.asarray(bv, np.float32) @ np.asarray(wo, np.float32)
    )
    out = np.empty((B, S, H), np.float32)
    for b in range(B):
        out[b] = (
            per_core_ot[b].T.astype(np.float32)
            + per_core_ot[B + b].T.astype(np.float32)
            + bo_eff
        )
    return out


def kernel(q, k, v, wq, bq, wk, bk, wv, bv, wo, bo, _trace=False):
    from concourse.bass_utils import run_bass_kernel_spmd

    nc = _get_nc()
    in_maps = make_in_maps(q, k, v, wq, wk, wv, wo, bq)
    res = run_bass_kernel_spmd(
        nc, in_maps, core_ids=list(range(NCORES)), trace=_trace
    )
    _CACHE["last_results"] = res
    outs = [res.results[c]["ot"] for c in range(NCORES)]
    return assemble(outs, bv, wo, bo)



# revision 37
# speedup vs baseline: 1.0241x; 1.0047x over previous
"""Multi-head attention (B=4,S=2048,H=1024,NH=16,D=64) on 8 trn2 cores.

Sharding: core c = (g, b) with g = c // 4 (head-group of 8 heads = 512 dims,
tensor parallel) and b = c % 4 (batch, data parallel). Each core computes a
partial output (its head-group's contribution to the final projection),
transposed: ot = (attn_out_g @ wo_g)^T of shape [H, S]. Host sums the two
group partials per batch and adds bias.

Math notes (host/device split):
  - k-proj bias bk drops out of softmax (adds a per-query constant along the
    key axis), so it is not applied on device.
  - v-proj bias bv commutes through normalized attention (rows of the score
    matrix sum to 1): its contribution is bv @ wo, folded into the output
    bias on the host.

On-device layout: everything is computed transposed (feature dim on
partitions, sequence on the free axis) so the softmax key-axis lands on
partitions. Scores S^T are built per head as K_h^T(stationary) x Q_h^T,
exp() runs on the scalar engine straight out of PSUM, and the ones-column
appended to V in the AV matmul yields the softmax denominators for free.
"""

import sys

if "/opt/trn_rl_repo" not in sys.path:
    sys.path.insert(0, "/opt/trn_rl_repo")

import numpy as np

B, S, H, NH, D = 4, 2048, 1024, 16, 64
G = 2  # head-group split across cores (tensor parallel axis)
GH = H // G  # 512 dims (8 heads) per group
NCORES = 8
SCALE = 1.0 / float(D) ** 0.5  # 1/8

KT = H // 128  # 8 contraction tiles for projections
MT = GH // 128  # 4 m-tiles = head pairs per group
NQC = S // 512  # 4 sequence chunks of 512
SQ = S // 128  # 16 key-sequence tiles
VW = D + 1  # 65: V columns + ones column per head

_CACHE = {}

# build-time tuning knobs (TimelineSim-swept)
CFG = {
    "xs_bufs": 24,
    "w_bufs": 24,
    "pt_bufs": 16,
    "mm_bufs": 2,
    "o_bufs": 2,
    "gpsimd_bcast": True,  # broadcast recip row via gpsimd (else DRAM round-trip)
}


def _build():
    import concourse.tile as tile
    from concourse import bacc, mybir

    F32 = mybir.dt.float32
    F32R = mybir.dt.float16  # all-f16 variant: f16 matmuls everywhere
    BF16 = mybir.dt.float16  # f16: same PE speed as bf16, 3 more mantissa bits
    AF = mybir.ActivationFunctionType
    OP = mybir.AluOpType

    nc = bacc.Bacc("TRN2", target_bir_lowering=False, debug=False)

    xq = nc.dram_tensor("xq", [H, S], F32R, kind="ExternalInput")
    xk = nc.dram_tensor("xk", [H, S], F32R, kind="ExternalInput")
    xv = nc.dram_tensor("xv", [H, S], F32R, kind="ExternalInput")
    wqd = nc.dram_tensor("wq", [H, GH], F32R, kind="ExternalInput")
    wkd = nc.dram_tensor("wk", [H, GH], F32R, kind="ExternalInput")
    wvd = nc.dram_tensor("wv", [H, GH], F32R, kind="ExternalInput")
    wod = nc.dram_tensor("wo", [GH, H], F32, kind="ExternalInput")
    bqd = nc.dram_tensor("bq", [GH], F32, kind="ExternalInput")
    otd = nc.dram_tensor("ot", [H, S], F32R, kind="ExternalOutput")

    with tile.TileContext(nc) as tc:
        with (
            tc.tile_pool(name="res", bufs=1) as res,
            tc.tile_pool(name="rot", bufs=2) as rot,
            tc.tile_pool(name="psmm", bufs=CFG["mm_bufs"], space="PSUM") as psmm,
            tc.tile_pool(name="pso", bufs=CFG["o_bufs"], space="PSUM") as pso,
            tc.tile_pool(name="dsc", bufs=4, space="DRAM") as dsc,
        ):
            # ---- residents ----
            qhT = [
                res.tile([128, S], F32R, tag=f"qhT{m}", name=f"qhT{m}")
                for m in range(MT)
            ]
            khT = [
                res.tile([128, S], F32R, tag=f"khT{m}", name=f"khT{m}")
                for m in range(MT)
            ]
            oT = [
                res.tile([128, S], BF16, tag=f"oT{t}", name=f"oT{t}")
                for t in range(MT)
            ]
            vaug = res.tile([128, SQ * 8 * VW], BF16, tag="vaug", name="vaug")
            wo_bf = [
                res.tile([128, H], BF16, tag=f"wob{t}", name=f"wob{t}")
                for t in range(MT)
            ]
            bq_sb = res.tile([128, MT], F32, tag="bqsb", name="bq_sb")

            # ---- constants / weights staging ----
            for m in range(MT):
                nc.sync.dma_start(
                    out=bq_sb[:, m : m + 1],
                    in_=bqd.ap()[m * 128 : (m + 1) * 128].rearrange(
                        "(p o) -> p o", o=1
                    ),
                )
            for t in range(MT):
                wos = rot.tile([128, H], F32, tag="wos", bufs=2, name=f"wos{t}")
                nc.sync.dma_start(out=wos, in_=wod.ap()[t * 128 : (t + 1) * 128, :])
                nc.vector.tensor_copy(wo_bf[t], wos)
            # ones columns of vaug (V slots are overwritten by the V proj)
            nc.vector.memset(vaug, 1.0)

            def load_w(wd):
                ws = []
                for kt in range(KT):
                    wt = rot.tile([128, GH], F32R, tag="w", bufs=CFG["w_bufs"], name=f"w{kt}")
                    nc.sync.dma_start(out=wt, in_=wd.ap()[kt * 128 : (kt + 1) * 128, :])
                    ws.append(wt)
                return ws

            def load_strips(xd, qc):
                xs = []
                for kt in range(KT):
                    st = rot.tile([128, 512], F32R, tag="xs", bufs=CFG["xs_bufs"], name=f"xs{kt}")
                    nc.sync.dma_start(
                        out=st,
                        in_=xd.ap()[
                            kt * 128 : (kt + 1) * 128, qc * 512 : (qc + 1) * 512
                        ],
                    )
                    xs.append(st)
                return xs

            # ---- chunked projections (issued piecemeal between/inside
            # attention blocks so they execute in the tensor engine's slack
            # while the scalar engine grinds exps) ----
            wv_sb = wk_sb = wq_sb = None
            strip_cache = {}

            def strips_for(key, xd, qc):
                if key not in strip_cache:
                    strip_cache[key] = load_strips(xd, qc)
                return strip_cache[key]

            def v_chunk(qc, sql):
                # one [128 seq, 512 dims] V-proj chunk -> vaug columns
                xs = strips_for(("v", qc), xv, qc)
                sq = qc * 4 + sql
                ps = psmm.tile([128, 512], F32, tag="mm", name=f"psv{sq}")
                for kt in range(KT):
                    nc.tensor.matmul(
                        ps,
                        lhsT=xs[kt][:, sql * 128 : (sql + 1) * 128],
                        rhs=wv_sb[kt],
                        start=(kt == 0),
                        stop=(kt == KT - 1),
                    )
                # single strided copy into the 8 stride-VW head slots: one
                # DVE op instead of 8 releases the PSUM slot ~1us sooner
                base = sq * 8 * VW
                nc.vector.tensor_copy(
                    vaug[:, base : base + 8 * VW].rearrange(
                        "p (h w) -> p h w", w=VW
                    )[:, :, 0:D],
                    ps.rearrange("p (h w) -> p h w", h=8),
                )

            def k_chunk(qc, m, pool=None):
                xs = strips_for(("k", qc), xk, qc)
                p = pool or psmm
                ps = p.tile([128, 512], F32, tag=("mm" if p is psmm else "o"), name=f"psk{m}")
                for kt in range(KT):
                    nc.tensor.matmul(
                        ps,
                        lhsT=wk_sb[kt][:, m * 128 : (m + 1) * 128],
                        rhs=xs[kt],
                        start=(kt == 0),
                        stop=(kt == KT - 1),
                    )
                nc.vector.tensor_copy(khT[m][:, qc * 512 : (qc + 1) * 512], ps)

            def q_chunk(qc, m, pool=None):
                xs = strips_for(("q", qc), xq, qc)
                p = pool or psmm
                ps = p.tile([128, 512], F32, tag=("mm" if p is psmm else "o"), name=f"psq{m}")
                for kt in range(KT):
                    nc.tensor.matmul(
                        ps,
                        lhsT=wq_sb[kt][:, m * 128 : (m + 1) * 128],
                        rhs=xs[kt],
                        start=(kt == 0),
                        stop=(kt == KT - 1),
                    )
                nc.vector.tensor_scalar(
                    qhT[m][:, qc * 512 : (qc + 1) * 512],
                    ps,
                    bq_sb[:, m : m + 1],
                    None,
                    OP.add,
                )

            def release_strips(key):
                strip_cache.pop(key, None)

            def lowprio(thunk, bump=1 << 20):
                # issue with priorities far in the future: the scheduler
                # dispatches these only into true slack — attention always
                # wins contested PSUM slots and engine time
                def run():
                    p = tc.cur_priority
                    tc.cur_priority = p + bump
                    thunk()
                    tc.cur_priority = p
                return run

            # ---- attention for head pair t over a 1024-wide query chunk ----
            # hooks: {kt: [thunk, ...]} — projection chunks issued at the top
            # of the kt iteration so they fill tensor slack without breaking
            # sequential dependency semantics
            def attention(t, qcp, hooks=None, pend_depth=3):
                q0 = qcp * 1024
                ps_o = [
                    pso.tile([VW, 1024], F32, tag="o", name=f"pso{hh}")
                    for hh in range(2)
                ]
                pending = []  # (kt, [pt_qch0, pt_qch1]) awaiting AV
                def emit_av(kt, pts):
                    for hh in range(2):
                        h_abs = 2 * t + hh
                        vbase = kt * 8 * VW + h_abs * VW
                        for qch in range(2):
                            nc.tensor.matmul(
                                ps_o[hh][:, qch * 512 : (qch + 1) * 512],
                                lhsT=vaug[:, vbase : vbase + VW],
                                rhs=pts[qch][:, hh * 512 : (hh + 1) * 512],
                                start=(kt == 0),
                                stop=(kt == SQ - 1),
                            )
                for kt in range(SQ):
                    if hooks and kt in hooks:
                        for thunk in hooks[kt]:
                            thunk()
                    # per query-half, BOTH heads' scores go into one shared
                    # PSUM tile (h0 -> bank 0 cols, h64 -> bank 1 cols): the
                    # pair becomes ready at the same instant, dispatches
                    # adjacently, and runs concurrently in disjoint PE
                    # row-groups (auto tile_position from base partition)
                    pts = []
                    for qch in range(2):
                        ps_s = psmm.tile([128, 1024], F32, tag="mm", name="pss")
                        for hh in range(2):
                            hp = 64 * hh
                            nc.tensor.matmul(
                                ps_s[:, hh * 512 : (hh + 1) * 512],
                                lhsT=khT[t][hp : hp + 64, kt * 128 : (kt + 1) * 128],
                                rhs=qhT[t][
                                    hp : hp + 64, q0 + qch * 512 : q0 + (qch + 1) * 512
                                ],
                                start=True,
                                stop=True,
                            )
                        pt_t = rot.tile([128, 1024], BF16, tag="pt", bufs=CFG["pt_bufs"], name="pt")
                        nc.scalar.activation(pt_t, ps_s, AF.Exp, scale=SCALE)
                        pts.append(pt_t)
                    pending.append((kt, pts))
                    if len(pending) > pend_depth:
                        emit_av(*pending.pop(0))
                for p in pending:
                    emit_av(*p)
                # normalize by the ones-column sums; heads stack on partitions
                for hh in range(2):
                    # drain the PSUM accumulator to SBUF immediately so the
                    # next head pair's AV can claim the banks while
                    # normalization is still in flight; the denom row DMAs to
                    # partition 0 (recip_approx_fast and partition_broadcast
                    # silently read partition 0 on HW whatever the AP says)
                    ou = rot.tile([VW, 1024], F32, tag="ou", bufs=2, name="ou")
                    nc.vector.tensor_copy(ou, ps_o[hh])
                    # the rest of the chain is latency-tolerant: issue at low
                    # priority so projection-chunk copies in the DVE FIFO
                    # (which gate PSUM slot releases) dispatch first
                    pri = tc.cur_priority
                    tc.cur_priority = pri + (1 << 20)
                    # rcp half lives at column 0 (partition_broadcast reads
                    # the tile base address)
                    dr = rot.tile([1, 2048], F32, tag="dr", bufs=2, name="dr")
                    den = dr[:, 1024:2048]
                    rcp = dr[:, 0:1024]
                    nc.sync.dma_start(out=den, in_=ou[D : D + 1, :])
                    nc.vector.reciprocal_approx_fast(rcp, den)
                    bc = rot.tile([64, 1024], F32, tag="bc", bufs=2, name="bc")
                    if CFG["gpsimd_bcast"]:
                        nc.gpsimd.partition_broadcast(bc, rcp)
                    else:
                        # DRAM round-trip (DRAM source DMA supports 0-stride
                        # partition reads)
                        sc = dsc.tile([1, 1024], F32, tag="sc", name="sc")
                        nc.sync.dma_start(out=sc, in_=rcp)
                        nc.sync.dma_start(
                            out=bc, in_=sc[0, :].partition_broadcast(64)
                        )
                    if hh == 0:
                        nc.vector.tensor_tensor(
                            oT[t][0:64, q0 : q0 + 1024],
                            ou[0:D, :],
                            bc,
                            OP.mult,
                        )
                    else:
                        # normalized h1 lands on partitions 0-63; DMA shifts it
                        # onto partitions 64-127 of the head-pair tile
                        otn = rot.tile([64, 1024], BF16, tag="otn", bufs=2, name="otn")
                        nc.vector.tensor_tensor(otn, ou[0:D, :], bc, OP.mult)
                        nc.sync.dma_start(
                            out=oT[t][64:128, q0 : q0 + 1024], in_=otn
                        )
                    tc.cur_priority = pri

            # ---- output projection for one 512-wide sequence chunk ----
            # f16 output halves the write traffic; at the tail the scalar
            # engine is idle so its HWDGE queue doubles the drain bandwidth
            def out_proj(qcc, spread=False, pool=None):
                for m in range(H // 128):
                    p = pool or (pso if m % 2 else psmm)
                    ps = p.tile([128, 512], F32, tag=("mm" if p is psmm else "o"), name=f"pso{m}")
                    for t in range(MT):
                        nc.tensor.matmul(
                            ps,
                            lhsT=wo_bf[t][:, m * 128 : (m + 1) * 128],
                            rhs=oT[t][:, qcc * 512 : (qcc + 1) * 512],
                            start=(t == 0),
                            stop=(t == MT - 1),
                        )
                    osb = rot.tile([128, 512], F32R, tag="osb", bufs=4, name="osb")
                    nc.vector.tensor_copy(osb, ps)
                    eng = nc.scalar if (spread and m % 2) else nc.sync
                    eng.dma_start(
                        out=otd.ap()[m * 128 : (m + 1) * 128, qcc * 512 : (qcc + 1) * 512],
                        in_=osb,
                    )

            # ---- main sequence: minimal prefix (K cols 0:1024 + Q m0),
            # then attention blocks with projection chunks interleaved via
            # hooks into the scalar-bound kt loops ----
            wk_sb = load_w(wkd)
            for qc in range(NQC):
                for m in range(MT):
                    k_chunk(qc, m, pool=(pso if m % 2 else psmm))
                release_strips(("k", qc))
            wq_sb = load_w(wqd)
            for qc in range(2):
                for m in range(MT):
                    q_chunk(qc, m, pool=(pso if m % 2 else psmm))
                release_strips(("q", qc))
            wv_sb = load_w(wvd)

            # block (0,0) hooks: V chunks paced just ahead of their AV
            # consumers, filling tensor slack in the scalar-bound kt loop
            hooks00 = {
                0: [lowprio(lambda: v_chunk(0, 0))],
                1: [lowprio(lambda: v_chunk(0, 1))],
                2: [lowprio(lambda: v_chunk(0, 2))],
                3: [lowprio(lambda: v_chunk(0, 3))],
                4: [lowprio(lambda: v_chunk(1, 0))],
                5: [lowprio(lambda: v_chunk(1, 1))],
                6: [lowprio(lambda: v_chunk(1, 2))],
                7: [lowprio(lambda: v_chunk(1, 3))],
                8: [lowprio(lambda: v_chunk(2, 0))],
                9: [lowprio(lambda: v_chunk(2, 1))],
                10: [lowprio(lambda: v_chunk(2, 2))],
                11: [lowprio(lambda: v_chunk(2, 3))],
                12: [lowprio(lambda: v_chunk(3, 0))],
                13: [lowprio(lambda: v_chunk(3, 1)), lowprio(lambda: v_chunk(3, 2))],
                14: [lowprio(lambda: v_chunk(3, 3))],
            }
            attention(0, 0, hooks00)
            attention(1, 0)
            attention(2, 0)
            attention(3, 0)
            for m in range(MT):
                q_chunk(2, m)
                q_chunk(3, m, pool=pso)
            attention(0, 1)
            out_proj(0)
            attention(1, 1)
            out_proj(1)
            attention(2, 1)
            attention(3, 1, pend_depth=1)
            # tail: all PSUM banks are free — run the two final chunks from
            # separate pools so four m-tiles pipeline concurrently
            out_proj(2, spread=True)
            out_proj(3, spread=True)

    nc.compile()
    return nc


def _get_nc():
    if "nc" not in _CACHE:
        _CACHE["nc"] = _build()
    return _CACHE["nc"]


def make_in_maps(q, k, v, wq, wk, wv, wo, bq):
    q = np.asarray(q, np.float32)
    k = np.asarray(k, np.float32)
    v = np.asarray(v, np.float32)
    in_maps = []
    for c in range(NCORES):
        g, b = divmod(c, B)
        sl = slice(g * GH, (g + 1) * GH)
        in_maps.append(
            {
                "xq": np.ascontiguousarray(q[b].T).astype(np.float16),
                "xk": np.ascontiguousarray(k[b].T).astype(np.float16),
                "xv": np.ascontiguousarray(v[b].T).astype(np.float16),
                "wq": np.ascontiguousarray(np.asarray(wq, np.float32)[:, sl]).astype(np.float16),
                "wk": np.ascontiguousarray(np.asarray(wk, np.float32)[:, sl]).astype(np.float16),
                "wv": np.ascontiguousarray(np.asarray(wv, np.float32)[:, sl]).astype(np.float16),
                "wo": np.ascontiguousarray(np.asarray(wo, np.float32)[sl, :]),
                "bq": np.ascontiguousarray(np.asarray(bq, np.float32)[sl]),
            }
        )
    return in_maps


def assemble(per_core_ot, bv, wo, bo):
    bo_eff = (
        np.asarray(bo, np.float32)
        + np.asarray(bv, np.float32) @ np.asarray(wo, np.float32)
    )
    out = np.empty((B, S, H), np.float32)
    for b in range(B):
        out[b] = (
            per_core_ot[b].T.astype(np.float32)
            + per_core_ot[B + b].T.astype(np.float32)
            + bo_eff
        )
    return out


def kernel(q, k, v, wq, bq, wk, bk, wv, bv, wo, bo, _trace=False):
    from concourse.bass_utils import run_bass_kernel_spmd

    nc = _get_nc()
    in_maps = make_in_maps(q, k, v, wq, wk, wv, wo, bq)
    res = run_bass_kernel_spmd(
        nc, in_maps, core_ids=list(range(NCORES)), trace=_trace
    )
    _CACHE["last_results"] = res
    outs = [res.results[c]["ot"] for c in range(NCORES)]
    return assemble(outs, bv, wo, bo)

